# revision 7
# baseline (speedup 1.0000x reference)
"""GCN layers (3x GCNConv + PReLU + residual + BatchNorm) on 8 TRN2 NeuronCores.

Full-input contract: kernel(**inputs) takes unsharded numpy arrays and returns
the full [50000, 64] float32 output.

Key restructuring vs the naive scheme: GCN aggregation is linear, so
  agg = segsum(w_e * (h W)[src]) = segsum(w_e * h[src]) @ W
and BatchNorm is a per-feature affine h = gs*p + gb, so
  segsum(w_e * h[src]) = gs * segsum(w_e * p[src]) + gb * deg_w[dst].
Therefore the gather table per layer is the RAW pre-BN activation p:
 - no matmul before the table AllGather,
 - the BN-stats AllReduce overlaps the entire next aggregation phase,
 - layer 0's table (= x) is precomputed on host and replicated (no
   collective at all in layer 0).

Sharding: nodes in 8 contiguous ranges (dst-sharded edges). Per layer:
 1. AllGather node-major bf16 table p [50176, 128] (64 valid cols) split in
    two pieces A/B (keeps gather idx in int16; overlaps AG-B with gathers-A)
 2. stream edges: dma_gather 256B rows table[src] -> SBUF; scatter matrices
    S (one nonzero w_e per edge row) are built ON DEVICE from a persistent
    [col, w] bf16 table via iota==col compare (2 DVE ops per 6144-edge
    chunk); per-128-edge subchunk matmuls agg^T[blk] += msg^T @ S (PSUM)
 3. transform + epilogue feature-major: hagg = gs*ragg + gb*degw (bf16),
    agg^T = W^T @ hagg^T, +bias, PReLU, +residual, with BN stats
    accumulated for free via accum_out; tiny stats AllReduce fully
    overlapped with the next layer's table AllGather + gathers.
"""

import os
import numpy as np

N_NODES = 50000
D = 64
L = 3
BN_EPS = 1e-5
N_CORES = 8
GCHUNK = 6144           # edge slots per gather chunk (48 subchunks of 128)
BLKN = 128              # dst nodes per aggregation block (S columns)
IDX_LIMIT = 32768       # int16 gather index range

LAST_RUN = {}


# ----------------------------------------------------------------------------
# Host-side preprocessing
# ----------------------------------------------------------------------------

def _wrap16(flat, slots):
    """Edge-slot array -> [128, slots/16] int16 'wrapped' index layout."""
    a = flat.reshape(slots // 16, 16).T.astype(np.int16)
    return np.tile(a, (8, 1))


def _preprocess(x, edge_src, edge_dst, edge_weight, W, b, prelu_a,
                bn_gamma, bn_beta, n_cores, nsh, gchunk, blkn):
    import ml_dtypes
    bf16 = ml_dtypes.bfloat16

    n = x.shape[0]
    d = x.shape[1]
    nt = (nsh + 127) // 128
    npad = nt * 128
    subc = gchunk // 128
    nblk = npad // blkn
    asplit = (IDX_LIMIT // n_cores) // 128 * 128     # 4096 local rows -> A
    bsplit = npad - asplit                            # 2176 local rows -> B
    rows_a = n_cores * asplit
    rows_b = n_cores * bsplit

    src = np.asarray(edge_src).astype(np.int64)
    dst = np.asarray(edge_dst).astype(np.int64)
    w = np.asarray(edge_weight).astype(np.float32)
    x = np.asarray(x).astype(np.float32)

    s_rank = src // nsh
    s_loc = src % nsh
    inA = s_loc < asplit
    idxA = s_rank * asplit + s_loc
    idxB = s_rank * bsplit + (s_loc - asplit)
    shard = dst // nsh
    dst_local = dst % nsh

    streams = []
    for sel, tix, trows in ((inA, idxA, rows_a), (~inA, idxB, rows_b)):
        per_core_edges = []
        cnts = []
        for r in range(n_cores):
            m = (shard == r) & sel
            per_core_edges.append((tix[m], dst_local[m], w[m]))
            cnts.append(np.bincount(dst_local[m] // blkn, minlength=nblk))
        nsub = np.zeros(nblk, np.int64)
        for c in cnts:
            nsub = np.maximum(nsub, (c + 127) // 128)
        sub_off = np.concatenate([[0], np.cumsum(nsub)])
        total_subs = int(sub_off[-1])
        nch = max(1, (total_subs + subc - 1) // subc)
        padded_subs = nch * subc
        slots = padded_subs * 128

        sched = []
        for blk in range(nblk):
            for j in range(int(nsub[blk])):
                gsub = int(sub_off[blk]) + j
                sched.append((gsub // subc, gsub % subc, blk,
                              j == 0, j == int(nsub[blk]) - 1))

        per_core = []
        for r in range(n_cores):
            ti, dl, wr = per_core_edges[r]
            blk = dl // blkn
            col = dl % blkn
            order = np.argsort(blk, kind="stable")
            ti, wr, blk, col = (a[order] for a in (ti, wr, blk, col))
            cnt = cnts[r]
            starts = np.concatenate([[0], np.cumsum(cnt)])
            pos = np.arange(len(ti)) - starts[blk]
            gsub = sub_off[blk] + pos // 128
            row = pos % 128
            slot = gsub * 128 + row
            rng_pad = np.random.default_rng(12345 + r)
            idx = rng_pad.integers(0, trows, slots)
            idx[total_subs * 128:] = -1   # uniform tail dummies: no descriptors
            idx[slot] = ti
            colw = np.zeros((padded_subs, 128, 2), np.float32)
            colw[gsub, row, 0] = col
            colw[gsub, row, 1] = wr
            colw_t = np.ascontiguousarray(colw.transpose(1, 0, 2)).astype(bf16)
            per_core.append((_wrap16(idx, slots), colw_t))
        streams.append(dict(nch=nch, slots=slots, sched=sched,
                            padded_subs=padded_subs,
                            total_subs=total_subs, per_core=per_core))

    # layer-0 gather tables (= x), replicated on every core
    tbl0A = np.zeros((rows_a, 128), np.float32)
    tbl0B = np.zeros((rows_b, 128), np.float32)
    for r in range(n_cores):
        tbl0A[r * asplit:(r + 1) * asplit, :d] = x[r * nsh:r * nsh + asplit]
        nb_real = nsh - asplit
        tbl0B[r * bsplit:r * bsplit + nb_real, :d] = \
            x[r * nsh + asplit:(r + 1) * nsh]
    tbl0A = tbl0A.astype(bf16)
    tbl0B = tbl0B.astype(bf16)

    bT = np.ascontiguousarray(np.asarray(b, np.float32).T)
    gammaT = np.ascontiguousarray(np.asarray(bn_gamma, np.float32).T)
    betaT = np.ascontiguousarray(np.asarray(bn_beta, np.float32).T)
    prelu_rep = np.tile(np.asarray(prelu_a, np.float32).reshape(1, L),
                        (128, 1))
    Wbf = np.ascontiguousarray(np.asarray(W, np.float32)).astype(bf16)

    in_maps = []
    for r in range(n_cores):
        m = (shard == r)
        degw = np.bincount(dst_local[m], weights=w[m],
                           minlength=nsh).astype(np.float32)
        degw_pad = np.zeros(npad, np.float32)
        degw_pad[:nsh] = degw
        degw_rep = np.tile(degw_pad.reshape(1, npad), (d, 1)).astype(bf16)
        in_maps.append({
            "tbl0A": tbl0A,
            "tbl0B": tbl0B,
            "Wbf": Wbf,
            "bT": bT,
            "gammaT": gammaT,
            "betaT": betaT,
            "prelu_rep": prelu_rep,
            "degw": degw_rep,
            "srcA": streams[0]["per_core"][r][0],
            "colwA": streams[0]["per_core"][r][1],
            "srcB": streams[1]["per_core"][r][0],
            "colwB": streams[1]["per_core"][r][1],
        })

    cfg = dict(n_cores=n_cores, nsh=nsh, d=d, nt=nt, npad=npad,
               gchunk=gchunk, subc=subc, blkn=blkn, nblk=nblk,
               asplit=asplit, bsplit=bsplit, rows_a=rows_a, rows_b=rows_b,
               n_nodes=n,
               nchA=streams[0]["nch"], slotsA=streams[0]["slots"],
               schedA=streams[0]["sched"], subsA=streams[0]["total_subs"],
               psubsA=streams[0]["padded_subs"],
               nchB=streams[1]["nch"], slotsB=streams[1]["slots"],
               schedB=streams[1]["sched"], subsB=streams[1]["total_subs"],
               psubsB=streams[1]["padded_subs"])
    return in_maps, cfg


# ----------------------------------------------------------------------------
# Device program
# ----------------------------------------------------------------------------

def _build_nc(cfg):
    import concourse.bacc as bacc
    import concourse.tile as tile
    import concourse.mybir as mybir
    from concourse import library_config
    from concourse.masks import make_identity

    fp32 = mybir.dt.float32
    bf16 = mybir.dt.bfloat16
    i16 = mybir.dt.int16
    i32 = mybir.dt.int32
    Alu = mybir.AluOpType
    Ax = mybir.AxisListType

    n_cores = cfg["n_cores"]
    nsh, d, nt, npad = cfg["nsh"], cfg["d"], cfg["nt"], cfg["npad"]
    gchunk, subc = cfg["gchunk"], cfg["subc"]
    blkn, nblk = cfg["blkn"], cfg["nblk"]
    asplit, bsplit = cfg["asplit"], cfg["bsplit"]
    rows_a, rows_b = cfg["rows_a"], cfg["rows_b"]
    n_nodes = cfg["n_nodes"]
    slotsA, slotsB = cfg["slotsA"], cfg["slotsB"]
    nchA, nchB = cfg["nchA"], cfg["nchB"]
    psubsA, psubsB = cfg["psubsA"], cfg["psubsB"]
    i16s = gchunk // 16
    nec = (npad + 511) // 512   # 512-col epilogue chunks

    def by_chunk(sched, nch):
        per = [[] for _ in range(nch)]
        for (c, j, blk, st, sp) in sched:
            per[c].append((j, blk, st, sp))
        return per

    schedA = by_chunk(cfg["schedA"], nchA)
    schedB = by_chunk(cfg["schedB"], nchB)

    nc = bacc.Bacc(None, target_bir_lowering=False, debug=False)

    tbl0A = nc.declare_dram_parameter("tbl0A", [rows_a, 128], bf16, isOutput=False)
    tbl0B = nc.declare_dram_parameter("tbl0B", [rows_b, 128], bf16, isOutput=False)
    Wbf_in = nc.declare_dram_parameter("Wbf", [L, d, d], bf16, isOutput=False)
    bT_in = nc.declare_dram_parameter("bT", [d, L], fp32, isOutput=False)
    gammaT_in = nc.declare_dram_parameter("gammaT", [d, L], fp32, isOutput=False)
    betaT_in = nc.declare_dram_parameter("betaT", [d, L], fp32, isOutput=False)
    prelu_in = nc.declare_dram_parameter("prelu_rep", [128, L], fp32, isOutput=False)
    degw_in = nc.declare_dram_parameter("degw", [d, npad], bf16, isOutput=False)
    srcA = nc.declare_dram_parameter("srcA", [128, slotsA // 16], i16, isOutput=False)
    colwA_in = nc.declare_dram_parameter("colwA", [128, psubsA * 2], bf16, isOutput=False)
    srcB = nc.declare_dram_parameter("srcB", [128, slotsB // 16], i16, isOutput=False)
    colwB_in = nc.declare_dram_parameter("colwB", [128, psubsB * 2], bf16, isOutput=False)
    out_ext = nc.declare_dram_parameter("out", [npad, d], fp32, isOutput=True)

    with tile.TileContext(nc) as tc:
        with (
            tc.tile_pool(name="const", bufs=1) as cpool,
            tc.tile_pool(name="state", bufs=1) as spool,
            tc.tile_pool(name="meta", bufs=1) as epool,
            tc.tile_pool(name="work", bufs=2) as wpool,
            tc.tile_pool(name="rows", bufs=1) as rpool,
            tc.tile_pool(name="msg", bufs=2) as mpool,
            tc.tile_pool(name="smat", bufs=2) as stpool,
            tc.tile_pool(name="ps", bufs=2, space="PSUM") as ppool,
            tc.tile_pool(name="psw", bufs=2, space="PSUM") as wppool,
            tc.tile_pool(name="psagg", bufs=2, space="PSUM") as apool,
            tc.tile_pool(name="dram", bufs=1, space="DRAM") as dpool,
        ):
            ragg = spool.tile([d, npad], fp32, tag="ragg")
            p_sb = spool.tile([d, npad], fp32, tag="p")
            h_sb = spool.tile([d, npad], fp32, tag="h")
            staging = spool.tile([128, nt, 128], bf16, tag="stg")
            sumacc = spool.tile([d, 16], fp32, tag="sumacc")
            sqacc = spool.tile([d, 16], fp32, tag="sqacc")
            stat_sb = spool.tile([d, 2], fp32, tag="stat")
            stat2_sb = spool.tile([d, 2], fp32, tag="stat2")

            ident = cpool.tile([d, d], fp32, tag="ident")
            W_sb = cpool.tile([d, L * d], bf16, tag="Wsb")
            bT_sb = cpool.tile([d, L], fp32, tag="bT")
            gaT_sb = cpool.tile([d, L], fp32, tag="gaT")
            beT_sb = cpool.tile([d, L], fp32, tag="beT")
            prelu_sb = cpool.tile([128, L], fp32, tag="prelu")
            degw_sb = cpool.tile([d, npad], bf16, tag="degw")
            iota_i = cpool.tile([128, 128], i32, tag="iotai")
            iota_sb = cpool.tile([128, 128], bf16, tag="iota")

            iA_sb = epool.tile([128, slotsA // 16], i16, tag="iA")
            iB_sb = epool.tile([128, slotsB // 16], i16, tag="iB")
            cwA_sb = epool.tile([128, psubsA, 2], bf16, tag="cwA")
            cwB_sb = epool.tile([128, psubsB, 2], bf16, tag="cwB")

            bounceA = dpool.tile([asplit, 128], bf16, tag="bA")
            bounceB = dpool.tile([bsplit, 128], bf16, tag="bB")
            tblA_sh = [dpool.tile([rows_a, 128], bf16, tag=f"tA{i}",
                                  name=f"tblA_sh{i}", addr_space="Shared")
                       for i in range(L - 1)]
            tblB_sh = [dpool.tile([rows_b, 128], bf16, tag=f"tB{i}",
                                  name=f"tblB_sh{i}", addr_space="Shared")
                       for i in range(L - 1)]
            stats_in = dpool.tile([2, d], fp32, tag="sin")
            stats_out = dpool.tile([2, d], fp32, tag="sout")

            nc.sync.dma_start(iA_sb[:], srcA[:])
            nc.sync.dma_start(iB_sb[:], srcB[:])
            nc.sync.dma_start(
                cwA_sb[:], colwA_in[:].rearrange("p (s t) -> p s t", t=2))
            nc.sync.dma_start(
                cwB_sb[:], colwB_in[:].rearrange("p (s t) -> p s t", t=2))
            for i in range(L):
                nc.sync.dma_start(W_sb[:, i * d:(i + 1) * d], Wbf_in[i, :, :])
            nc.sync.dma_start(bT_sb[:], bT_in[:])
            nc.sync.dma_start(gaT_sb[:], gammaT_in[:])
            nc.sync.dma_start(beT_sb[:], betaT_in[:])
            nc.sync.dma_start(prelu_sb[:], prelu_in[:])
            nc.sync.dma_start(degw_sb[:], degw_in[:])
            make_identity(nc, ident[:])
            nc.gpsimd.iota(iota_i[:], pattern=[[1, 128]], base=0,
                           channel_multiplier=0)
            nc.vector.tensor_copy(iota_sb[:], iota_i[:])
            nc.vector.memset(staging[:], 0.0)
            nc.gpsimd.load_library(library_config.mlp)

            def build_S(S, cw_sb, c):
                iota_b = iota_sb[:].unsqueeze(1).broadcast_to((128, subc, 128))
                colv = cw_sb[:, c * subc:(c + 1) * subc, 0:1] \
                    .broadcast_to((128, subc, 128))
                wv = cw_sb[:, c * subc:(c + 1) * subc, 1:2] \
                    .broadcast_to((128, subc, 128))
                nc.vector.tensor_tensor(out=S[:], in0=iota_b, in1=colv,
                                        op=Alu.is_equal)
                nc.vector.tensor_tensor(out=S[:], in0=S[:], in1=wv,
                                        op=Alu.mult)

            for i in range(L):
                tblA_ap = tbl0A[:] if i == 0 else tblA_sh[i - 1][:]
                tblB_ap = tbl0B[:] if i == 0 else tblB_sh[i - 1][:]

                # ---- aggregate raw table rows: ragg^T[blk] += msg^T @ S ----
                nc.vector.memset(ragg[:], 0.0)
                agg_ps = {}
                for (tbl_ap, idx_sb, cw_sb, nch, sched, tsubs) in (
                        (tblA_ap, iA_sb, cwA_sb, nchA, schedA, cfg["subsA"]),
                        (tblB_ap, iB_sb, cwB_sb, nchB, schedB, cfg["subsB"])):
                    for c in range(nch):
                        msg = mpool.tile([128, subc, 128], bf16, tag="msg")
                        nvalid = min(gchunk,
                                     max(0, tsubs * 128 - c * gchunk))
                        nc.gpsimd.dma_gather(
                            msg[:], tbl_ap,
                            idx_sb[:, c * i16s:(c + 1) * i16s],
                            num_idxs=gchunk, num_idxs_reg=nvalid,
                            elem_size=128, queue_num=0,
                            single_packet=False)
                        S_sb = stpool.tile([128, subc, blkn], bf16, tag="S")
                        build_S(S_sb, cw_sb, c)
                        for (j, blk, st, sp) in sched[c]:
                            if st:
                                agg_ps[blk] = apool.tile(
                                    [d, blkn], fp32, name="aggps", tag="aggps")
                            nc.tensor.matmul(agg_ps[blk][:],
                                             lhsT=msg[:, j, 0:d],
                                             rhs=S_sb[:, j, :],
                                             start=st, stop=sp)
                            if sp:
                                lo = blk * blkn
                                nc.vector.tensor_tensor(
                                    out=ragg[:, lo:lo + blkn],
                                    in0=ragg[:, lo:lo + blkn],
                                    in1=agg_ps.pop(blk)[:], op=Alu.add)

                # ---- consume last layer's BN stats (fully overlapped AR) ----
                gs_c = rpool.tile([d, 1], fp32, tag="gsc")
                gb_c = rpool.tile([d, 1], fp32, tag="gbc")
                if i > 0:
                    mean_c = rpool.tile([d, 1], fp32, tag="meanc")
                    var_c = rpool.tile([d, 1], fp32, tag="varc")
                    tmp_c = rpool.tile([d, 1], fp32, tag="tmpc")
                    inv_n = 1.0 / float(n_nodes)
                    nc.sync.dma_start(stat2_sb[:],
                                      stats_out[:].rearrange("s d -> d s"))
                    nc.vector.tensor_scalar_mul(mean_c[:], stat2_sb[:, 0:1],
                                                inv_n)
                    nc.vector.tensor_scalar_mul(var_c[:], stat2_sb[:, 1:2],
                                                inv_n)
                    nc.vector.tensor_tensor(out=tmp_c[:], in0=mean_c[:],
                                            in1=mean_c[:], op=Alu.mult)
                    nc.vector.tensor_tensor(out=var_c[:], in0=var_c[:],
                                            in1=tmp_c[:], op=Alu.subtract)
                    nc.vector.tensor_scalar_add(var_c[:], var_c[:], BN_EPS)
                    nc.scalar.activation(tmp_c[:], var_c[:],
                                         mybir.ActivationFunctionType.Sqrt)
                    nc.vector.reciprocal(var_c[:], tmp_c[:])
                    nc.vector.tensor_tensor(out=gs_c[:],
                                            in0=gaT_sb[:, i - 1:i],
                                            in1=var_c[:], op=Alu.mult)
                    nc.vector.tensor_tensor(out=tmp_c[:], in0=mean_c[:],
                                            in1=gs_c[:], op=Alu.mult)
                    nc.vector.tensor_tensor(out=gb_c[:],
                                            in0=beT_sb[:, i - 1:i],
                                            in1=tmp_c[:], op=Alu.subtract)
                    # h_{i-1} = gs*p + gb (residual input for this layer)
                    nc.vector.tensor_scalar(out=h_sb[:], in0=p_sb[:],
                                            scalar1=gs_c[:], scalar2=gb_c[:],
                                            op0=Alu.mult, op1=Alu.add)

                # ---- transform + epilogue, 512-col chunks ------------------
                for ch in range(nec):
                    lo = ch * 512
                    hi = min(npad, lo + 512)
                    cw = hi - lo
                    hbf = wpool.tile([d, 512], bf16, tag="hbf")
                    if i == 0:
                        nc.vector.tensor_copy(hbf[:, :cw], ragg[:, lo:hi])
                    else:
                        dgb = wpool.tile([d, 512], fp32, tag="dgb")
                        nc.vector.tensor_scalar_mul(
                            dgb[:, :cw], degw_sb[:, lo:hi], gb_c[:])
                        nc.vector.scalar_tensor_tensor(
                            out=hbf[:, :cw], in0=ragg[:, lo:hi],
                            scalar=gs_c[:], in1=dgb[:, :cw],
                            op0=Alu.mult, op1=Alu.add)
                    wps = wppool.tile([d, 512], fp32, tag="wps")
                    nc.tensor.matmul(wps[:, :cw],
                                     lhsT=W_sb[:, i * d:(i + 1) * d],
                                     rhs=hbf[:, :cw], start=True, stop=True)
                    t_ch = wpool.tile([d, 512], fp32, tag="tch")
                    u_ch = wpool.tile([d, 512], fp32, tag="uch")
                    nc.vector.tensor_scalar_add(t_ch[:, :cw], wps[:, :cw],
                                                bT_sb[:, i:i + 1])
                    nc.vector.tensor_scalar_mul(u_ch[:, :cw], t_ch[:, :cw],
                                                prelu_sb[:d, i:i + 1])
                    if i > 0:
                        nc.vector.tensor_tensor(out=t_ch[:, :cw],
                                                in0=t_ch[:, :cw],
                                                in1=u_ch[:, :cw], op=Alu.max)
                        nc.vector.scalar_tensor_tensor(
                            out=p_sb[:, lo:hi], in0=t_ch[:, :cw], scalar=0.0,
                            op0=Alu.add, in1=h_sb[:, lo:hi], op1=Alu.add,
                            accum_out=sumacc[:, ch:ch + 1])
                    else:
                        nc.vector.scalar_tensor_tensor(
                            out=p_sb[:, lo:hi], in0=t_ch[:, :cw], scalar=0.0,
                            op0=Alu.add, in1=u_ch[:, :cw], op1=Alu.max,
                            accum_out=sumacc[:, ch:ch + 1])
                    if ch == nec - 1 and npad > nsh:
                        # zero pad cols so stats sums see exact zeros
                        nc.vector.memset(p_sb[:, nsh:npad], 0.0)
                    sq = wpool.tile([d, 512], fp32, tag="sqch")
                    nc.vector.scalar_tensor_tensor(
                        out=sq[:, :cw], in0=p_sb[:, lo:hi], scalar=0.0,
                        op0=Alu.add, in1=p_sb[:, lo:hi], op1=Alu.mult,
                        accum_out=sqacc[:, ch:ch + 1])
                    # next-layer gather table: transpose p chunk to node-major
                    if i < L - 1:
                        for t in range(lo // 128, hi // 128):
                            tr_ps = ppool.tile([128, d], fp32, tag="trps")
                            nc.tensor.transpose(
                                tr_ps[:], p_sb[:, t * 128:(t + 1) * 128],
                                ident[:])
                            nc.vector.tensor_copy(staging[:, t, 0:d],
                                                  tr_ps[:])
                    # kick off AllGathers as soon as each half is staged
                    if i < L - 1 and hi == asplit:
                        ta = asplit // 128
                        nc.sync.dma_start(
                            bounceA[:].rearrange("(t p) m -> p t m", p=128),
                            staging[:, 0:ta, :])
                        nc.gpsimd.collective_compute(
                            "AllGather", mybir.AluOpType.bypass,
                            replica_groups=[list(range(n_cores))],
                            ins=[bounceA.opt()], outs=[tblA_sh[i].opt()])
                    if i < L - 1 and hi == npad:
                        ta = asplit // 128
                        nc.sync.dma_start(
                            bounceB[:].rearrange("(t p) m -> p t m", p=128),
                            staging[:, ta:nt, :])
                        nc.gpsimd.collective_compute(
                            "AllGather", mybir.AluOpType.bypass,
                            replica_groups=[list(range(n_cores))],
                            ins=[bounceB.opt()], outs=[tblB_sh[i].opt()])

                # ---- stats reduce + AllReduce (consumed next layer) --------
                nc.vector.reduce_sum(stat_sb[:, 0:1], sumacc[:, 0:nec],
                                     axis=Ax.X)
                nc.vector.reduce_sum(stat_sb[:, 1:2], sqacc[:, 0:nec],
                                     axis=Ax.X)
                nc.sync.dma_start(stats_in[:].rearrange("s d -> d s"),
                                  stat_sb[:])
                nc.gpsimd.collective_compute(
                    "AllReduce", mybir.AluOpType.add,
                    replica_groups=[list(range(n_cores))],
                    ins=[stats_in.opt()], outs=[stats_out.opt()])

            # ---- final BN + output --------------------------------------
            gs_f = rpool.tile([d, 1], fp32, tag="gsf")
            gb_f = rpool.tile([d, 1], fp32, tag="gbf")
            mean_f = rpool.tile([d, 1], fp32, tag="meanf")
            var_f = rpool.tile([d, 1], fp32, tag="varf")
            tmp_f = rpool.tile([d, 1], fp32, tag="tmpf")
            inv_n = 1.0 / float(n_nodes)
            nc.sync.dma_start(stat2_sb[:],
                              stats_out[:].rearrange("s d -> d s"))
            nc.vector.tensor_scalar_mul(mean_f[:], stat2_sb[:, 0:1], inv_n)
            nc.vector.tensor_scalar_mul(var_f[:], stat2_sb[:, 1:2], inv_n)
            nc.vector.tensor_tensor(out=tmp_f[:], in0=mean_f[:],
                                    in1=mean_f[:], op=Alu.mult)
            nc.vector.tensor_tensor(out=var_f[:], in0=var_f[:],
                                    in1=tmp_f[:], op=Alu.subtract)
            nc.vector.tensor_scalar_add(var_f[:], var_f[:], BN_EPS)
            nc.scalar.activation(tmp_f[:], var_f[:],
                                 mybir.ActivationFunctionType.Sqrt)
            nc.vector.reciprocal(var_f[:], tmp_f[:])
            nc.vector.tensor_tensor(out=gs_f[:], in0=gaT_sb[:, L - 1:L],
                                    in1=var_f[:], op=Alu.mult)
            nc.vector.tensor_tensor(out=tmp_f[:], in0=mean_f[:],
                                    in1=gs_f[:], op=Alu.mult)
            nc.vector.tensor_tensor(out=gb_f[:], in0=beT_sb[:, L - 1:L],
                                    in1=tmp_f[:], op=Alu.subtract)
            nc.vector.tensor_scalar(out=h_sb[:], in0=p_sb[:],
                                    scalar1=gs_f[:], scalar2=gb_f[:],
                                    op0=Alu.mult, op1=Alu.add)

            outv = staging[:].bitcast(fp32)   # [128, nt, 64] fp32 view
            for t in range(nt):
                tr_ps = ppool.tile([128, d], fp32, tag="trps")
                nc.tensor.transpose(tr_ps[:], h_sb[:, t * 128:(t + 1) * 128],
                                    ident[:])
                nc.vector.tensor_copy(outv[:, t, :], tr_ps[:])
            nc.sync.dma_start(out_ext[:].rearrange("(t p) d -> p t d", p=128),
                              outv)
    nc.compile()
    return nc


# ----------------------------------------------------------------------------
# Entry point
# ----------------------------------------------------------------------------

def kernel(x, edge_src, edge_dst, edge_weight, W, b, prelu_a,
           bn_gamma, bn_beta):
    from concourse.bass_utils import run_bass_kernel_spmd

    x = np.asarray(x)
    n = x.shape[0]
    nsh = n // N_CORES
    in_maps, cfg = _preprocess(x, edge_src, edge_dst, edge_weight, W, b,
                               prelu_a, bn_gamma, bn_beta,
                               N_CORES, nsh, GCHUNK, BLKN)
    nc = _build_nc(cfg)
    trace = bool(int(os.environ.get("GCN_TRACE", "0")))
    if trace:
        try:
            import antenv.axon_hooks  # noqa: F401
        except ImportError:
            trace = False
    res = run_bass_kernel_spmd(nc, in_maps, core_ids=list(range(N_CORES)),
                               trace=trace)
    LAST_RUN["results"] = res
    LAST_RUN["cfg"] = cfg
    LAST_RUN["nc"] = nc
    LAST_RUN["in_maps"] = in_maps
    out = np.concatenate(
        [res.results[r]["out"][:nsh] for r in range(N_CORES)], axis=0)
    return out.astype(np.float32)


def measure_exec_ns(nc, in_maps, n_reps=10):
    """Steady-state device-time estimate: pre-staged device inputs; marginal
    (slope) wall time of k back-to-back NEFF executions, amortizing the
    axon tunnel dispatch overhead."""
    import time
    import jax
    import concourse.mybir as mybir
    from jax.sharding import Mesh, PartitionSpec, NamedSharding
    from jax.experimental.shard_map import shard_map
    from concourse import bass2jax

    n_cores = len(in_maps)
    partition_name = (nc.partition_id_tensor.name
                      if nc.partition_id_tensor else None)
    in_names, out_names, out_avals = [], [], []
    for alloc in nc.m.functions[0].allocations:
        if not isinstance(alloc, mybir.MemoryLocationSet):
            continue
        name = alloc.memorylocations[0].name
        if alloc.kind == "ExternalInput":
            if name != partition_name:
                in_names.append(name)
        elif alloc.kind == "ExternalOutput":
            out_names.append(name)
            out_avals.append(jax.core.ShapedArray(
                tuple(alloc.tensor_shape), mybir.dt.np(alloc.dtype)))
    n_params = len(in_names)
    all_in = list(in_names) + list(out_names)
    if partition_name is not None:
        all_in.append(partition_name)

    def _body(*args):
        operands = list(args)
        if partition_name is not None:
            operands.append(bass2jax.partition_id_tensor())
        outs = bass2jax._bass_exec_p.bind(
            *operands, out_avals=tuple(out_avals), in_names=tuple(all_in),
            out_names=tuple(out_names), lowering_input_output_aliases=(),
            sim_require_finite=True, sim_require_nnan=True, nc=nc)
        return tuple(outs)

    devices = jax.devices()[:n_cores]
    mesh = Mesh(np.asarray(devices), ("core",))
    nin = n_params + len(out_names)
    fn = jax.jit(shard_map(_body, mesh=mesh,
                           in_specs=(PartitionSpec("core"),) * nin,
                           out_specs=(PartitionSpec("core"),) * len(out_names),
                           check_rep=False))
    sh = NamedSharding(mesh, PartitionSpec("core"))
    dev_in = [jax.device_put(
        np.concatenate([np.asarray(in_maps[c][k]) for c in range(n_cores)],
                       axis=0), sh) for k in in_names]
    dev_zero = [jax.device_put(
        np.zeros((n_cores * a.shape[0], *a.shape[1:]), a.dtype), sh)
        for a in out_avals]
    out = fn(*dev_in, *dev_zero)
    jax.block_until_ready(out)

    def best_of(k, reps):
        best = 1e9
        for _ in range(reps):
            t0 = time.perf_counter()
            last = None
            for _ in range(k):
                last = fn(*dev_in, *dev_zero)
            jax.block_until_ready(last)
            best = min(best, time.perf_counter() - t0)
        return best

    # slopes between amortized queue depths; contention only adds time,
    # so the minimum pairwise marginal is the least-contaminated estimate
    t8 = best_of(8, 4)
    t16 = best_of(16, 4)
    t32 = best_of(32, 4)
    cands = [(t32 - t8) / 24, (t32 - t16) / 16, (t16 - t8) / 8]
    pos = [c for c in cands if c > 0]
    marginal = min(pos) if pos else abs((t32 - t8) / 24)
    times = [t8, t16, t32]
    return int(marginal * 1e9), times
